# revision 44
# baseline (speedup 1.0000x reference)
"""Causal self-attention (B=4, T=2048, C=768, H=12) on 8 NeuronCores — v5.

Sharding: core <-> (batch b = core//2, heads h0 = 6*(core%2) .. h0+5).
Each core computes its 6 heads' attention plus the partial output projection;
the host sums the two half-head partials per batch.

v5 over v4: fp8(e4m3) DoubleRow matmuls halve/quarter the PE cost of QKV and
scores, and the exp work is split ACT/DVE:
  * QKV in fp8 DR: hi pass x8*W8 (chunk-pair slots) for q,k; v adds a cross
    pass with slots (x8*W8r, x8r*W8) so v reaches ~f16 quality (W is scaled
    x64 on host so fp8's min-normal doesn't eat N(0,0.02) weights; the 1/64
    rides the PSUM->SBUF conversion copies).
  * Scores in fp8 DR: lhsT = K-block broadcast into both k-tile slots
    (stride-0), rhs = (q8, q8r) residual slots -> q at ~f16 quality free.
    The 1/sqrt(d) folds into the exp scale operand.
  * exp split: most windows on ACT (activation Exp, scale=0.125); a tunable
    subset on DVE via Schraudolph (i16(a*s+b) bitcast f16, minimax bias,
    ~±3% -> ~1.3e-2 worst-case output error at 50% coverage).
  * Diagonal masks on GPSIMD (tensor_mul with 0/1 triangle, f16).
  * PV/normalize/transposes/projection unchanged from v4 (f16).
Schedule: 12-group pipeline with a k-step-paced deferred-PE work queue
(QKV pieces, PV blocks, norms, projections) sized to each k-step's exp
cover so the in-order PE queue never starves ACT; projections carry
tile_wait_until floors from an ACT-paced logical clock so the tile list
scheduler cannot hoist them into bubbles ahead of their norm->DMA-
transpose chains (an 8us priority inversion otherwise); schraudolph
windows pinned to sub1 so the sub0 score stream that paces ACT never
waits on a DVE-held PSUM buffer.
Result: 134.1us modeled (vs 153.6us for v4): ACT ~99us busy (74%),
DVE 93us, PE 81us, Pool 42us; the first diag k-step of each group exps
its full tile in one instr (the below-diagonal gap is never read by PV).
Remaining idle = ~5us DMA prologue, ~4.7us drain tail, and ~0.6us
score-ring bubbles after each DVE-exp'd window (PSUM's 8 banks cannot
fit a deeper score ring). Tried and rejected: dropping the q8r residual
+33% schraudolph (135.6us, err 1.11e-2 - DVE is not the binding path),
norm muls on ACT (150.9us - ACT's in-order queue delays exps), extra
schraudolph windows in groups 6-8 (+0.2us - bubbles offset the gain).
"""

import numpy as np
import ml_dtypes

import concourse.bass as bass
import concourse.mybir as mybir
import concourse.tile as tile
from concourse import bacc
from concourse.bass_utils import run_bass_kernel_spmd

F32 = mybir.dt.float32
F16 = mybir.dt.float16
F8 = mybir.dt.float8e4
I16 = mybir.dt.int16
NP8 = ml_dtypes.float8_e4m3
DR = mybir.MatmulPerfMode.DoubleRow

T = 2048
C = 768
D = 64
HPC = 6          # heads per core
NCC = 6          # C / 128
NT = 16          # T / 128
NJ = 4           # T / 512
EXP = mybir.ActivationFunctionType.Exp
QSC = 0.125      # 1/sqrt(D)
WIN = 1.0 / 64.0  # undo the x64 host W scaling in psum->sbuf conversions
# Schraudolph: f16 bitcast of i16(rint(a*s + b)), minimax-centered
SCH_A = QSC * 1024.0 / float(np.log(2.0))
SCH_B = 15360.5 - 45.0


def _emit(nc, tc, x8w, w8qk, wvhi, wvcr, wp, out):
    from contextlib import ExitStack
    with ExitStack() as ctx:
        pp = ctx.enter_context(tc.tile_pool(name="persist", bufs=1))

        # fp8 q/k: Qt[p] holds (q8 | q8r) halves, Kt[p] plain
        Qt = [pp.tile([128, 2 * T], F8, tag=f"qt{m}", name=f"qt{m}") for m in range(3)]
        Kt = [pp.tile([128, T], F8, tag=f"kt{m}", name=f"kt{m}") for m in range(3)]
        vaug = [pp.tile([128, HPC * (D + 1)], F16, tag=f"v{t}", name=f"vaug{t}") for t in range(NT)]
        OF = [pp.tile([128, T], F16, tag=f"of{p}", name=f"of{p}") for p in range(3)]
        tri = pp.tile([128, 128], F16, tag="tri", name="tri01")

        xtw = ctx.enter_context(tc.tile_pool(name="xtw", bufs=1))
        epl = ctx.enter_context(tc.tile_pool(name="epool", bufs=34))
        attsm = ctx.enter_context(tc.tile_pool(name="attsm", bufs=1))
        osb = ctx.enter_context(tc.tile_pool(name="outsb", bufs=6))
        # PSUM: scores 4 banks + PV 2 banks + shared qkv/proj ring 2 banks
        sp = ctx.enter_context(tc.tile_pool(name="spsum", bufs=2, space="PSUM"))
        pvp = ctx.enter_context(tc.tile_pool(name="pvpsum", bufs=1, space="PSUM"))
        mm = ctx.enter_context(tc.tile_pool(name="mmpsum", bufs=2, space="PSUM"))

        warm = attsm.tile([1, 8], F32, tag="warm", name="warmup")
        nc.vector.memset(warm[:], 0.0)
        nc.scalar.activation(warm[0:1, 0:8], warm[0:1, 0:8], EXP)
        # 0/1 upper-triangle (keep iff col >= row)
        nc.vector.memset(tri[:], 1.0)
        nc.gpsimd.affine_select(
            out=tri[:], in_=tri[:], pattern=[[1, 128]],
            compare_op=mybir.AluOpType.is_ge, fill=0.0,
            base=0, channel_multiplier=-1)
        # identity permutation matrix for tail PE transposes
        iden = pp.tile([128, 128], F16, tag="iden", name="identity")
        nc.vector.memset(iden[:], 1.0)
        nc.gpsimd.affine_select(
            out=iden[:], in_=iden[:], pattern=[[1, 128]],
            compare_op=mybir.AluOpType.is_equal, fill=0.0,
            base=0, channel_multiplier=-1)
        # dummy matmul chain: ramps the PE p-state (3us to full clock) while
        # the prologue DMAs are in flight; output is never read
        pewarm = mm.tile([128, 512], F32, tag="mm", name="pewarm")
        for r in range(8):
            nc.tensor.matmul(out=pewarm[:], lhsT=iden[:], rhs=OF[0][:, 0:512],
                             start=(r == 0), stop=(r == 7))

        # -------- DVE schraudolph window picker (full windows only) --------
        # Only ever sub1 windows, every other k-step: the sub0 stream that
        # paces ACT then never sits behind a DVE-held score buffer.
        def use_dve(n, k, sub):
            if sub != 1:
                return False
            return (k % 2) == 0 or (n in (9, 10) and (k % 4) == 1)

        def emit_exp(dst, src, on_dve):
            if on_dve:
                nc.vector.tensor_scalar(
                    out=dst.bitcast(I16), in0=src,
                    scalar1=SCH_A, scalar2=SCH_B,
                    op0=mybir.AluOpType.mult, op1=mybir.AluOpType.add)
            else:
                nc.scalar.activation(dst, src, EXP, scale=QSC)

        def gen_ph1(n, p, j, es):
            """generator: DR scores + exp (+ diag mask); yields the k-step's
            ACT exp ns and PE score cycles after each k-step."""
            ni = 4 * j + 4
            for k in range(ni // 2):
                i0 = 2 * k
                ss = {}
                act_ns = 0.0
                cover_ns = 0.0
                score_cyc = 0
                for sub in (0, 1):
                    ss[sub] = sp.tile([128, 1024], F32, tag="s", name=f"s{p}{j}{k}{sub}")
                # sub-major order: sub0's window completes without waiting on
                # sub1's buffer (which frees only when exp(k-1,sub1) ends)
                for sub in (0, 1):
                    b0 = 64 * sub
                    for idx in (0, 1):
                        i = i0 + idx
                        isl = slice(128 * i, 128 * (i + 1))
                        off = max(0, 128 * i - 512 * j)
                        lo = 512 * idx
                        score_cyc += (512 - off) // 2
                        nc.tensor.matmul(
                            out=ss[sub][:, lo + off:lo + 512],
                            lhsT=Kt[p][b0:b0 + 64, isl].unsqueeze(1)
                                 .broadcast_to((64, 2, 128)),
                            rhs=Qt[p][b0:b0 + 64, :]
                                .rearrange("q (s t) -> q s t", s=2)
                                [:, :, 512 * j + off:512 * (j + 1)],
                            start=True, stop=True, perf_mode=DR,
                        )
                for sub in (0, 1):
                    e = epl.tile([128, 1024], F16, tag="e", name=f"e{p}{j}{k}{sub}")
                    if i0 + 1 < 4 * j:
                        if use_dve(n, k, sub):
                            emit_exp(e[:], ss[sub][:], True)
                            cover_ns += 1192
                        else:
                            emit_exp(e[:], ss[sub][:], False)
                            act_ns += 1024 * 0.833 + 185
                    elif i0 == 4 * j:
                        # first diag k-step: idx0 full + idx1 off=128; exp the
                        # whole tile in one instr (the 128-col gap is below-
                        # diagonal garbage that PV never reads)
                        emit_exp(e[:], ss[sub][:], False)
                        act_ns += 1024 * 0.833 + 185
                    else:
                        # last diag k-step: sub1's two windows ride DVE (the
                        # following group-boundary k0-sub1 is DVE-bound too,
                        # so no extra score-ring bubble)
                        for idx in (0, 1):
                            i = i0 + idx
                            off = max(0, 128 * i - 512 * j)
                            lo = 512 * idx
                            emit_exp(e[:, lo + off:lo + 512],
                                     ss[sub][:, lo + off:lo + 512], sub == 1)
                            if sub == 1:
                                cover_ns += (512 - off) * 1.0417 + 125
                            else:
                                act_ns += (512 - off) * 0.833 + 185
                    for idx in (0, 1):
                        i = i0 + idx
                        if i >= 4 * j:
                            off = 128 * i - 512 * j
                            win = e[:, 512 * idx + off:512 * idx + off + 128]
                            nc.gpsimd.tensor_mul(win, win, tri[:])
                    es[(sub, k)] = e
                yield act_ns, cover_ns, score_cyc

        def drain(g):
            for _ in g:
                pass

        def step(g):
            y = next(g, None)
            if y is not None:
                clk[0] += y[0]

        pv_tiles = {}

        def pv_alloc(p, j):
            pva = pvp.tile([128, 512], F32, tag="pvA", name=f"pva{p}{j}")
            pvb = pvp.tile([128, 512], F32, tag="pvB", name=f"pvb{p}{j}")
            pv_tiles[(p, j)] = (pva, pvb)
            return (pva, pva, pvb, pvb)

        def pv_block(p, j, es, bank, i):
            """PV matmuls for tk-block i of group (p,j)."""
            lo = 512 * (i % 2)
            kt = i // 2
            for tp in range(max(0, i - 4 * j), 4):
                for sub in (0, 1):
                    h = 2 * p + sub
                    kk = 2 * (tp % 2) + sub
                    nc.tensor.matmul(
                        out=bank[tp][:, 65 * kk:65 * kk + 65],
                        lhsT=es[(sub, kt)][:, lo + 128 * tp:lo + 128 * tp + 128],
                        rhs=vaug[i][:, 65 * h:65 * h + 65],
                        start=(i == 0 and tp % 2 == 0 and sub == 0),
                        stop=(i == 4 * j + tp and tp % 2 == 1 and sub == 1),
                    )

        def gen_pv(p, j, es):
            bank = pv_alloc(p, j)
            for i in range(4 * j + 4):
                pv_block(p, j, es, bank, i)

        def norm_tp(p, j, bank, ra, tp, name):
            o2 = attsm.tile([128, 128], F16, tag=f"o2n{tp}", bufs=3, name=name)
            for sub in (0, 1):
                kk = 2 * (tp % 2) + sub
                rcol = 4 * (tp // 2) + kk
                nc.vector.tensor_scalar_mul(
                    o2[:, 64 * sub:64 * sub + 64],
                    bank[tp][:, 65 * kk:65 * kk + 64],
                    ra[:, rcol:rcol + 1])
            t = 4 * j + tp
            nc.sync.dma_start_transpose(
                out=OF[p][:, 128 * t:128 * (t + 1)], in_=o2[:])

        def norm_recips(p, j):
            pva, pvb = pv_tiles.pop((p, j))
            bank = (pva, pva, pvb, pvb)
            ra = attsm.tile([128, 8], F32, tag="ra", bufs=3, name=f"ra{p}{j}")
            nc.vector.reciprocal(
                ra[:, 0:4].rearrange("p (k o) -> p k o", o=1),
                pva[:, 0:260].rearrange("p (k c) -> p k c", c=65)[:, :, 64:65])
            nc.vector.reciprocal(
                ra[:, 4:8].rearrange("p (k o) -> p k o", o=1),
                pvb[:, 0:260].rearrange("p (k c) -> p k c", c=65)[:, :, 64:65])
            return bank, ra

        def gen_norm(p, j):
            bank, ra = norm_recips(p, j)
            for tp in range(4):
                norm_tp(p, j, bank, ra, tp, f"o2n{p}{j}{tp}")

        # ACT-paced logical clock (ns): floors deferred-work start times so
        # the tile list-scheduler cannot hoist them into too-early PE bubbles
        # (its CoreSim model underestimates the DVE->DMA norm chain).
        clk = [5000.0]

        def emit_proj_eo(t, eo, el):
            ob = osb.tile([128, 512], F32, tag="ob", name=f"ob{t}_{eo}")
            ps = mm.tile([128, 512], F32, tag="mm", name=f"pps{t}_{eo}")
            with tc.tile_wait_until(clk[0] / 1e6):
                for p in range(3):
                    nc.tensor.matmul(
                        out=ps[:, 0:el],
                        lhsT=OF[p][:, 128 * t:128 * (t + 1)],
                        rhs=wp_b[:, 768 * p + eo:768 * p + eo + el],
                        start=(p == 0), stop=(p == 2),
                    )
                nc.vector.tensor_copy(ob[:, 0:el], ps[:, 0:el])
            nc.sync.dma_start(out=out[128 * t:128 * (t + 1), eo:eo + el],
                              in_=ob[:, 0:el])

        def emit_proj_t(t):
            for eo, el in ((0, 512), (512, 256)):
                emit_proj_eo(t, eo, el)

        # ---------------- input DMAs (progressive) ----------------
        xw = pp.tile([128, 4 * 6144], F8, tag="xw", name="xw")
        w8qk_t = pp.tile([128, 4608], F8, tag="w8qk", name="w8qk")
        wvhi_t = pp.tile([128, 2304], F8, tag="wvhi", name="wvhi")
        wvcr_t = pp.tile([128, 4608], F8, tag="wvcr", name="wvcr")
        wp_b = xtw.tile([128, 3 * C], F16, tag="wpb", name="wpb")

        # first-chunk-first DMA order: the prologue's qk(0,0)/qk(3,0) chains
        # start as soon as chunk-pair a of window 0 and its weights land
        for a in range(3):
            nc.sync.dma_start(out=w8qk_t[:, 1536 * a:1536 * (a + 1)],
                              in_=w8qk[:, 1536 * a:1536 * (a + 1)])
            nc.sync.dma_start(out=xw[:, 2048 * a:2048 * (a + 1)],
                              in_=x8w[:, 2048 * a:2048 * (a + 1)])
        nc.sync.dma_start(out=wvhi_t[:], in_=wvhi)
        nc.sync.dma_start(out=wvcr_t[:], in_=wvcr)
        nc.sync.dma_start(out=xw[:, 6144:12288], in_=x8w[:, 6144:12288])
        nc.sync.dma_start(
            out=wp_b[:].rearrange("p (c w) -> p c w", w=C),
            in_=wp[:, :].rearrange("(c p2) w -> p2 c w", c=3))
        nc.sync.dma_start(out=xw[:, 12288:18432], in_=x8w[:, 12288:18432])
        nc.sync.dma_start(out=xw[:, 18432:24576], in_=x8w[:, 18432:24576])

        def xw4(j):
            # [128, c(6), h(2), t(512)] view of window j
            return xw[:, 6144 * j:6144 * (j + 1)].rearrange(
                "p (c h t) -> p c h t", c=6, h=2)

        def emit_qk(m, j, on_act=False):
            """q/k tile (m<3: q pair m; m>=3: k pair m-3), window j: hi-only.
            on_act: route the plain conversions through the (prologue-idle)
            ACT engine instead of DVE."""
            ps = mm.tile([128, 512], F32, tag="mm", name=f"qkps{m}_{j}")
            x4 = xw4(j)
            for a in range(3):
                nc.tensor.matmul(
                    out=ps[:],
                    lhsT=w8qk_t[:, 1536 * a:1536 * (a + 1)]
                        .rearrange("p (s w) -> p s w", s=2)
                        [:, :, 128 * m:128 * m + 128],
                    rhs=x4[:, 2 * a:2 * a + 2, 0:1, :].squeeze(2),
                    start=(a == 0), stop=(a == 2), perf_mode=DR,
                )
            if m < 3:
                if on_act:
                    nc.scalar.mul(Qt[m][:, 512 * j:512 * (j + 1)], ps[:], WIN)
                else:
                    nc.vector.tensor_scalar_mul(
                        Qt[m][:, 512 * j:512 * (j + 1)], ps[:], WIN)
                nc.vector.scalar_tensor_tensor(
                    out=Qt[m][:, T + 512 * j:T + 512 * (j + 1)],
                    in0=ps[:], scalar=WIN,
                    in1=Qt[m][:, 512 * j:512 * (j + 1)],
                    op0=mybir.AluOpType.mult,
                    op1=mybir.AluOpType.subtract)
            else:
                if on_act:
                    nc.scalar.mul(Kt[m - 3][:, 512 * j:512 * (j + 1)], ps[:], WIN)
                else:
                    nc.vector.tensor_scalar_mul(
                        Kt[m - 3][:, 512 * j:512 * (j + 1)], ps[:], WIN)

        def emit_v(t):
            nc.gpsimd.memset(vaug[t][:], 1.0)
            ps = mm.tile([128, 512], F32, tag="mm", name=f"vps{t}")[:, 0:HPC * D]
            x4 = xw4(t // 4)
            tb = 128 * (t % 4)
            for a in range(3):
                nc.tensor.matmul(
                    out=ps,
                    lhsT=x4[:, 2 * a:2 * a + 2, 0:1, tb:tb + 128].squeeze(2),
                    rhs=wvhi_t[:, 768 * a:768 * (a + 1)]
                        .rearrange("p (s w) -> p s w", s=2),
                    start=(a == 0), stop=False, perf_mode=DR,
                )
            for c in range(NCC):
                nc.tensor.matmul(
                    out=ps,
                    lhsT=x4[:, c:c + 1, :, tb:tb + 128].squeeze(1),
                    rhs=wvcr_t[:, 768 * c:768 * (c + 1)]
                        .rearrange("p (s w) -> p s w", s=2),
                    start=False, stop=(c == NCC - 1), perf_mode=DR,
                )
            dst = vaug[t].rearrange("p (h c) -> p h c", c=D + 1)[:, :, 0:D]
            src = ps.rearrange("p (h c) -> p h c", c=D)
            nc.vector.tensor_scalar_mul(dst, src, WIN)

        # ------ qkv piece distribution: each hoisted group's qk tiles land
        # one iteration before the hoist needs them --------------------------
        PIECES = {
            0: [("qk", 1, 0), ("qk", 4, 0), ("v", 0), ("v", 1)],
            1: [("qk", 2, 0), ("qk", 5, 0), ("v", 2), ("v", 3)],
            2: [("qk", 0, 1), ("qk", 3, 1), ("v", 4)],
            3: [("qk", 1, 1), ("qk", 4, 1), ("v", 5), ("v", 6)],
            4: [("qk", 2, 1), ("qk", 5, 1), ("v", 7)],
            5: [("qk", 0, 2), ("qk", 3, 2), ("v", 8)],
            6: [("qk", 1, 2), ("qk", 4, 2), ("v", 9), ("v", 10)],
            7: [("qk", 2, 2), ("qk", 5, 2), ("v", 11)],
            8: [("qk", 0, 3), ("qk", 3, 3), ("v", 12)],
            9: [("qk", 1, 3), ("qk", 4, 3), ("v", 13), ("v", 14)],
            10: [("qk", 2, 3), ("qk", 5, 3), ("v", 15)],
            11: [],
        }

        def emit_pieces_qk(n):
            for pc in PIECES[n]:
                if pc[0] == "qk":
                    emit_qk(pc[1], pc[2])

        def emit_pieces_v(n):
            for pc in PIECES[n]:
                if pc[0] == "v":
                    emit_v(pc[1])

        # ---------------- unified pipeline --------------------------------
        # Deferred PE work queue: tasks (cost-in-PE-cycles, closure) slotted
        # into the ACT-covered window of each score k-step so the in-order PE
        # queue never blocks the score stream that feeds ACT.
        import collections as _co
        pe_q = _co.deque()

        def pump(budget):
            while pe_q and budget > 0 and pe_q[0][0] <= budget:
                cost, fn = pe_q.popleft()
                fn()
                budget -= cost

        def pump_all():
            while pe_q:
                pe_q.popleft()[1]()

        def pump_k(y):
            if y is None:
                return
            act_ns, cover_ns, score_cyc = y
            clk[0] += act_ns
            pump(max(0, int((act_ns + cover_ns) * 2.04) - score_cyc))

        def enq_piece(pc):
            if pc[0] == "qk":
                m, jj = pc[1], pc[2]
                pe_q.append((800, lambda m=m, jj=jj: emit_qk(m, jj)))
            else:
                t = pc[1]
                pe_q.append((1800, lambda t=t: emit_v(t)))

        def enq_pv(p, j, es):
            box = {}
            for i in range(4 * j + 4):
                ntp = 4 - max(0, i - 4 * j)

                def run(i=i):
                    if i == 0:
                        box["bank"] = pv_alloc(p, j)
                    pv_block(p, j, es, box["bank"], i)

                pe_q.append((ntp * 130, run))

        def enq_proj_t(t):
            for eo, el in ((0, 512), (512, 256)):
                pe_q.append((3 * el, lambda t=t, eo=eo, el=el:
                             emit_proj_eo(t, eo, el)))

        groups = [(p, j) for j in range(NJ) for p in range(3)]
        E = {}

        # prologue: q0/k0 for window 0 (conversions on the idle ACT engine)
        emit_qk(0, 0, on_act=True)
        emit_qk(3, 0, on_act=True)
        E[0] = {}
        cur = gen_ph1(0, 0, 0, E[0])
        step(cur)                      # k0 of group 0
        for n, (p, j) in enumerate(groups):
            if n < 11:
                for y in cur:          # rest of ph1(g_n), pumping PE work
                    pump_k(y)
                E[n + 1] = {}
                cur = gen_ph1(n + 1, *groups[n + 1], E[n + 1])
                # next group's q/k tiles must be emitted before its hoisted k0
                for pc in PIECES[n]:
                    if pc[0] == "qk":
                        emit_qk(pc[1], pc[2])
                pump_k(next(cur, None))  # k0 of g_{n+1}
                for pc in PIECES[n]:
                    if pc[0] == "v":
                        enq_piece(pc)
                if n >= 2:
                    gp, gj = groups[n - 2]
                    pe_q.append((0, lambda gp=gp, gj=gj: gen_norm(gp, gj)))
                if n >= 1:
                    enq_pv(*groups[n - 1], E[n - 1])
                # projections spread over the later iterations, each at least
                # one iteration after its level's last norm has popped (the
                # norm->transpose->OF chain has ~2.4us DMA latency)
                if n in (6, 7, 9):
                    for t in {6: (0, 1), 7: (2, 3), 9: (4, 5, 6, 7)}[n]:
                        enq_proj_t(t)
            else:
                # last group (2,3): fuse pv(2,3) into its own score/exp loop,
                # and normalize/project each PSUM bank as soon as it stops
                step(cur)              # k1
                pump_all()             # drain any deferred PE backlog
                step(cur)              # k2
                gen_norm(*groups[n - 2])
                gen_pv(*groups[n - 1], E[n - 1])        # pv(1,3)
                gen_norm(*groups[n - 1])                # norm(1,3)
                bank = pv_alloc(p, j)

                def tail_norm_bank(btile, tps):
                    for tp in tps:
                        ra = attsm.tile([128, 8], F32, tag="ra", bufs=3,
                                        name=f"raT{tp}")
                        k0_ = 2 * (tp % 2)
                        nc.vector.reciprocal(
                            ra[:, k0_:k0_ + 2].rearrange("p (k o) -> p k o", o=1),
                            btile[:, 65 * k0_:65 * k0_ + 130].rearrange(
                                "p (k c) -> p k c", c=65)[:, :, 64:65])
                        o2 = attsm.tile([128, 128], F16, tag=f"o2n{tp}", bufs=3,
                                        name=f"o2nT{tp}")
                        for sub in (0, 1):
                            kk = 2 * (tp % 2) + sub
                            if sub == 0:
                                # ACT is idle after the last exp: split the two
                                # scale-converts across ACT and DVE
                                nc.scalar.mul(
                                    o2[:, 0:64],
                                    btile[:, 65 * kk:65 * kk + 64],
                                    ra[:, kk:kk + 1])
                            else:
                                nc.vector.tensor_scalar_mul(
                                    o2[:, 64 * sub:64 * sub + 64],
                                    btile[:, 65 * kk:65 * kk + 64],
                                    ra[:, kk:kk + 1])
                        t = 4 * j + tp
                        # PE transpose (latency ~0.1us vs ~2.4us DMA xbar);
                        # copies ride the now-idle ACT engine
                        tps_ = mm.tile([128, 512], F32, tag="mm",
                                       name=f"tps{tp}")[:, 0:64].bitcast(F16)
                        nc.tensor.matmul(out=tps_, lhsT=o2[:], rhs=iden[:],
                                         is_transpose=True)
                        nc.scalar.copy(OF[p][:, 128 * t:128 * (t + 1)], tps_)
                        ob = osb.tile([128, C], F32, tag="ob", name=f"obT{t}")
                        for eo, el in ((0, 512), (512, 256)):
                            ps = mm.tile([128, 512], F32, tag="mm",
                                         name=f"ppsT{t}_{eo}")
                            for pr in range(3):
                                nc.tensor.matmul(
                                    out=ps[:, 0:el],
                                    lhsT=OF[pr][:, 128 * t:128 * (t + 1)],
                                    rhs=wp_b[:, 768 * pr + eo:768 * pr + eo + el],
                                    start=(pr == 0), stop=(pr == 2),
                                )
                            if eo == 0:
                                nc.scalar.copy(ob[:, 0:512], ps[:, 0:512])
                            else:
                                nc.vector.tensor_copy(ob[:, eo:eo + el],
                                                      ps[:, 0:el])
                            nc.sync.dma_start(
                                out=out[128 * t:128 * (t + 1), eo:eo + el],
                                in_=ob[:, eo:eo + el])

                pvb_ = bank[2]

                def pv_block_a(i):
                    # [tq,65]-orientation blocks for t'=0,1 only (bank A)
                    lo = 512 * (i % 2)
                    kt = i // 2
                    for tp in range(max(0, i - 12), 2):
                        for sub in (0, 1):
                            h = 2 * p + sub
                            kk = 2 * tp + sub
                            nc.tensor.matmul(
                                out=bank[0][:, 65 * kk:65 * kk + 65],
                                lhsT=E[n][(sub, kt)][:, lo + 128 * tp:lo + 128 * tp + 128],
                                rhs=vaug[i][:, 65 * h:65 * h + 65],
                                start=(i == 0 and tp == 0 and sub == 0),
                                stop=(i == 12 + tp and tp == 1 and sub == 1),
                            )

                def pv_slab(i):
                    # t14/t15 in [65,tq] orientation: O^T slab [65, 2x256]
                    lo = 512 * (i % 2)
                    kt = i // 2
                    off = max(0, 128 * i - 1792)
                    for sub in (0, 1):
                        h = 2 * p + sub
                        nc.tensor.matmul(
                            out=pvb_[0:65, 256 * sub + off:256 * sub + 256],
                            lhsT=vaug[i][:, 65 * h:65 * h + 65],
                            rhs=E[n][(sub, kt)][:, lo + 256 + off:lo + 512],
                            start=(i == 0 and sub == 0),
                            stop=(i == 15 and sub == 1),
                        )

                nb = 0
                for k in range(3, 8):
                    step(cur)                           # k-step k
                    if 3 <= k <= 6:
                        emit_proj_t(k + 5)              # t8..t11
                    while nb <= 2 * k - 1:
                        pv_block_a(nb)
                        pv_slab(nb)
                        nb += 1
                        if nb == 14:                    # bank A (t' 0,1) stopped
                            tail_norm_bank(bank[0], (0, 1))
                for i in range(nb, 16):
                    pv_block_a(i)
                    pv_slab(i)
                pv_tiles.pop((p, j))
                # slab normalize: recip row + partition_broadcast + scale into
                # OF, then project t14/t15
                rrow = attsm.tile([1, 512], F32, tag="rrow", name="rrowT")
                rb = attsm.tile([64, 512], F32, tag="rbT", name="rbT")
                nc.vector.reciprocal(rrow[0:1, :], pvb_[64:65, :])
                nc.gpsimd.partition_broadcast(rb[0:64, :], rrow[0:1, :])
                for sub in (0, 1):
                    nc.vector.tensor_mul(
                        OF[p][64 * sub:64 * sub + 64, 1792:2048],
                        pvb_[0:64, 256 * sub:256 * sub + 256],
                        rb[0:64, 256 * sub:256 * sub + 256])
                for t in (14, 15):
                    ob = osb.tile([128, C], F32, tag="ob", name=f"obT{t}")
                    for eo, el in ((0, 512), (512, 256)):
                        ps = mm.tile([128, 512], F32, tag="mm",
                                     name=f"ppsT{t}_{eo}")
                        for pr in range(3):
                            nc.tensor.matmul(
                                out=ps[:, 0:el],
                                lhsT=OF[pr][:, 128 * t:128 * (t + 1)],
                                rhs=wp_b[:, 768 * pr + eo:768 * pr + eo + el],
                                start=(pr == 0), stop=(pr == 2),
                            )
                        if eo == 0:
                            nc.scalar.copy(ob[:, 0:512], ps[:, 0:512])
                        else:
                            nc.vector.tensor_copy(ob[:, eo:eo + el],
                                                  ps[:, 0:el])
                        nc.sync.dma_start(
                            out=out[128 * t:128 * (t + 1), eo:eo + el],
                            in_=ob[:, eo:eo + el])


_NC_CACHE = None


def build_nc():
    global _NC_CACHE
    if _NC_CACHE is not None:
        return _NC_CACHE
    nc = bacc.Bacc(trn_type="TRN2")
    x8w = nc.dram_tensor("x8w", [128, 24576], F8, kind="ExternalInput").ap()
    w8qk = nc.dram_tensor("w8qk", [128, 4608], F8, kind="ExternalInput").ap()
    wvhi = nc.dram_tensor("wvhi", [128, 2304], F8, kind="ExternalInput").ap()
    wvcr = nc.dram_tensor("wvcr", [128, 4608], F8, kind="ExternalInput").ap()
    wp = nc.dram_tensor("wp", [HPC * D, C], F16, kind="ExternalInput").ap()
    out = nc.dram_tensor("out", [T, C], F32, kind="ExternalOutput").ap()
    with tile.TileContext(nc) as tc:
        _emit(nc, tc, x8w, w8qk, wvhi, wvcr, wp, out)
    nc.compile()
    _NC_CACHE = nc
    return nc


def _f8(a):
    return np.asarray(a, dtype=NP8)


def make_in_maps(x, W_attn, W_proj):
    x = np.asarray(x, dtype=np.float32)
    W_attn = np.asarray(W_attn, dtype=np.float32)
    W_proj = np.asarray(W_proj, dtype=np.float32)
    W64 = 64.0 * W_attn
    W8 = _f8(W64)
    W8r = _f8(W64 - W8.astype(np.float32))
    in_maps = []
    for core in range(8):
        b = core // 2
        h0 = HPC * (core % 2)
        xT = np.ascontiguousarray(x[b].T)            # [C, T]
        x8 = _f8(xT)
        x8r = _f8(xT - x8.astype(np.float32))
        # x8w[p, 6144j + 1024c + 512h + t] = (x8,x8r)[h][128c+p, 512j+t]
        st = np.stack([x8, x8r], axis=0).reshape(2, 6, 128, 4, 512)
        x8w = np.ascontiguousarray(
            st.transpose(2, 3, 1, 0, 4).reshape(128, 24576))
        # q,k hi weights (no q pre-scale; 1/sqrt(d) folds into exp scale)
        qcols = W8[:, 64 * h0:64 * h0 + 384]
        kcols = W8[:, 768 + 64 * h0:768 + 64 * h0 + 384]
        wqk = np.concatenate([qcols.astype(np.float32),
                              kcols.astype(np.float32)], axis=1)
        w8qk = np.ascontiguousarray(
            _f8(wqk).reshape(3, 2, 128, 768).transpose(2, 0, 1, 3)
            .reshape(128, 4608))
        vhi = W8[:, 1536 + 64 * h0:1536 + 64 * h0 + 384]
        vlo = W8r[:, 1536 + 64 * h0:1536 + 64 * h0 + 384]
        wvhi = np.ascontiguousarray(
            vhi.reshape(3, 2, 128, 384).transpose(2, 0, 1, 3)
            .reshape(128, 2304))
        # cross tile per chunk c: slot0 = W8r_c (pairs x8), slot1 = W8_c (pairs x8r)
        wvcr = np.ascontiguousarray(
            np.stack([vlo.reshape(6, 128, 384), vhi.reshape(6, 128, 384)],
                     axis=1).transpose(2, 0, 1, 3).reshape(128, 4608))
        wp_ = np.ascontiguousarray(
            W_proj[64 * h0:64 * h0 + 384, :]).astype(np.float16)
        in_maps.append({"x8w": x8w, "w8qk": w8qk, "wvhi": wvhi,
                        "wvcr": wvcr, "wp": wp_})
    return in_maps


def kernel(x, W_attn, W_proj, _trace=False, _trace_kwargs=None):
    nc = build_nc()
    in_maps = make_in_maps(x, W_attn, W_proj)
    res = run_bass_kernel_spmd(nc, in_maps, list(range(8)), trace=_trace,
                               **(_trace_kwargs or {}))
    outs = [res.results[c]["out"] for c in range(8)]
    y = np.stack([outs[2 * b] + outs[2 * b + 1] for b in range(4)]).astype(np.float32)
    if _trace:
        return y, res
    return y


# revision 45
# speedup vs baseline: 1.0863x; 1.0863x over previous
"""Causal self-attention (B=4, T=2048, C=768, H=12) on 8 NeuronCores — v5.

Sharding: core <-> (batch b = core//2, heads h0 = 6*(core%2) .. h0+5).
Each core computes its 6 heads' attention plus the partial output projection;
the host sums the two half-head partials per batch.

v5 over v4: fp8(e4m3) DoubleRow matmuls halve/quarter the PE cost of QKV and
scores, and the exp work is split ACT/DVE:
  * QKV in fp8 DR: hi pass x8*W8 (chunk-pair slots) for q,k; v adds a cross
    pass with slots (x8*W8r, x8r*W8) so v reaches ~f16 quality (W is scaled
    x64 on host so fp8's min-normal doesn't eat N(0,0.02) weights; the 1/64
    rides the PSUM->SBUF conversion copies).
  * Scores in fp8 DR: lhsT = K-block broadcast into both k-tile slots
    (stride-0), rhs = (q8, q8r) residual slots -> q at ~f16 quality free.
    The 1/sqrt(d) folds into the exp scale operand.
  * exp split: most windows on ACT (activation Exp, scale=0.125); a tunable
    subset on DVE via Schraudolph (i16(a*s+b) bitcast f16, minimax bias,
    ~±3% -> ~1.3e-2 worst-case output error at 50% coverage).
  * Diagonal masks on GPSIMD (tensor_mul with 0/1 triangle, f16).
  * PV/normalize/transposes/projection unchanged from v4 (f16).
Schedule: 12-group pipeline with a k-step-paced deferred-PE work queue
(QKV pieces, PV blocks, norms, projections) sized to each k-step's exp
cover so the in-order PE queue never starves ACT; projections carry
tile_wait_until floors from an ACT-paced logical clock so the tile list
scheduler cannot hoist them into bubbles ahead of their norm->DMA-
transpose chains (an 8us priority inversion otherwise); schraudolph
windows pinned to sub1 so the sub0 score stream that paces ACT never
waits on a DVE-held PSUM buffer.
Result: 134.2us modeled (vs 153.6us for v4): ACT ~99us busy (74%),
DVE 93us, PE 81us, Pool 42us; the first diag k-step of each group exps
its full tile in one instr (the below-diagonal gap is never read by PV).
Remaining idle = ~5us DMA prologue, ~4.7us drain tail, and ~0.6us
score-ring bubbles after each DVE-exp'd window (PSUM's 8 banks cannot
fit a deeper score ring). Tried and rejected: dropping the q8r residual
+33% schraudolph (135.6us, err 1.11e-2 - DVE is not the binding path),
norm muls on ACT (150.9us - ACT's in-order queue delays exps).
"""

import numpy as np
import ml_dtypes

import concourse.bass as bass
import concourse.mybir as mybir
import concourse.tile as tile
from concourse import bacc
from concourse.bass_utils import run_bass_kernel_spmd

F32 = mybir.dt.float32
F16 = mybir.dt.float16
F8 = mybir.dt.float8e4
I16 = mybir.dt.int16
NP8 = ml_dtypes.float8_e4m3
DR = mybir.MatmulPerfMode.DoubleRow

T = 2048
C = 768
D = 64
HPC = 6          # heads per core
NCC = 6          # C / 128
NT = 16          # T / 128
NJ = 4           # T / 512
EXP = mybir.ActivationFunctionType.Exp
QSC = 0.125      # 1/sqrt(D)
WIN = 1.0 / 64.0  # undo the x64 host W scaling in psum->sbuf conversions
# Schraudolph: f16 bitcast of i16(rint(a*s + b)), minimax-centered
SCH_A = QSC * 1024.0 / float(np.log(2.0))
SCH_B = 15360.5 - 45.0


def _emit(nc, tc, x8w, w8qk, wvhi, wvcr, wp, out):
    from contextlib import ExitStack
    with ExitStack() as ctx:
        pp = ctx.enter_context(tc.tile_pool(name="persist", bufs=1))

        # fp8 q/k: Qt[p] holds (q8 | q8r) halves, Kt[p] plain
        Qt = [pp.tile([128, 2 * T], F8, tag=f"qt{m}", name=f"qt{m}") for m in range(3)]
        Kt = [pp.tile([128, T], F8, tag=f"kt{m}", name=f"kt{m}") for m in range(3)]
        vaug = [pp.tile([128, HPC * (D + 1)], F16, tag=f"v{t}", name=f"vaug{t}") for t in range(NT)]
        OF = [pp.tile([128, T], F16, tag=f"of{p}", name=f"of{p}") for p in range(3)]
        tri = pp.tile([128, 128], F16, tag="tri", name="tri01")

        xtw = ctx.enter_context(tc.tile_pool(name="xtw", bufs=1))
        epl = ctx.enter_context(tc.tile_pool(name="epool", bufs=34))
        attsm = ctx.enter_context(tc.tile_pool(name="attsm", bufs=1))
        osb = ctx.enter_context(tc.tile_pool(name="outsb", bufs=6))
        # PSUM: scores 4 banks + PV 2 banks + shared qkv/proj ring 2 banks
        sp = ctx.enter_context(tc.tile_pool(name="spsum", bufs=2, space="PSUM"))
        pvp = ctx.enter_context(tc.tile_pool(name="pvpsum", bufs=1, space="PSUM"))
        mm = ctx.enter_context(tc.tile_pool(name="mmpsum", bufs=2, space="PSUM"))

        warm = attsm.tile([1, 8], F32, tag="warm", name="warmup")
        nc.vector.memset(warm[:], 0.0)
        nc.scalar.activation(warm[0:1, 0:8], warm[0:1, 0:8], EXP)
        # 0/1 upper-triangle (keep iff col >= row)
        nc.vector.memset(tri[:], 1.0)
        nc.gpsimd.affine_select(
            out=tri[:], in_=tri[:], pattern=[[1, 128]],
            compare_op=mybir.AluOpType.is_ge, fill=0.0,
            base=0, channel_multiplier=-1)
        # identity permutation matrix for tail PE transposes
        iden = pp.tile([128, 128], F16, tag="iden", name="identity")
        nc.vector.memset(iden[:], 1.0)
        nc.gpsimd.affine_select(
            out=iden[:], in_=iden[:], pattern=[[1, 128]],
            compare_op=mybir.AluOpType.is_equal, fill=0.0,
            base=0, channel_multiplier=-1)
        # dummy matmul chain: ramps the PE p-state (3us to full clock) while
        # the prologue DMAs are in flight; output is never read
        pewarm = mm.tile([128, 512], F32, tag="mm", name="pewarm")
        for r in range(8):
            nc.tensor.matmul(out=pewarm[:], lhsT=iden[:], rhs=OF[0][:, 0:512],
                             start=(r == 0), stop=(r == 7))

        # -------- DVE schraudolph window picker (full windows only) --------
        # Only ever sub1 windows, every other k-step: the sub0 stream that
        # paces ACT then never sits behind a DVE-held score buffer.
        def use_dve(n, k, sub):
            if sub != 1:
                return False
            return (k % 2) == 0 or (n in (9, 10) and (k % 4) == 1)

        def emit_exp(dst, src, on_dve):
            if on_dve:
                nc.vector.tensor_scalar(
                    out=dst.bitcast(I16), in0=src,
                    scalar1=SCH_A, scalar2=SCH_B,
                    op0=mybir.AluOpType.mult, op1=mybir.AluOpType.add)
            else:
                nc.scalar.activation(dst, src, EXP, scale=QSC)

        def gen_ph1(n, p, j, es):
            """generator: DR scores + exp (+ diag mask); yields the k-step's
            ACT exp ns and PE score cycles after each k-step."""
            ni = 4 * j + 4
            for k in range(ni // 2):
                i0 = 2 * k
                ss = {}
                act_ns = 0.0
                cover_ns = 0.0
                score_cyc = 0
                for sub in (0, 1):
                    ss[sub] = sp.tile([128, 1024], F32, tag="s", name=f"s{p}{j}{k}{sub}")
                # sub-major order: sub0's window completes without waiting on
                # sub1's buffer (which frees only when exp(k-1,sub1) ends)
                for sub in (0, 1):
                    b0 = 64 * sub
                    for idx in (0, 1):
                        i = i0 + idx
                        isl = slice(128 * i, 128 * (i + 1))
                        off = max(0, 128 * i - 512 * j)
                        lo = 512 * idx
                        score_cyc += (512 - off) // 2
                        nc.tensor.matmul(
                            out=ss[sub][:, lo + off:lo + 512],
                            lhsT=Kt[p][b0:b0 + 64, isl].unsqueeze(1)
                                 .broadcast_to((64, 2, 128)),
                            rhs=Qt[p][b0:b0 + 64, :]
                                .rearrange("q (s t) -> q s t", s=2)
                                [:, :, 512 * j + off:512 * (j + 1)],
                            start=True, stop=True, perf_mode=DR,
                        )
                for sub in (0, 1):
                    e = epl.tile([128, 1024], F16, tag="e", name=f"e{p}{j}{k}{sub}")
                    if i0 + 1 < 4 * j:
                        if use_dve(n, k, sub):
                            emit_exp(e[:], ss[sub][:], True)
                            cover_ns += 1192
                        else:
                            emit_exp(e[:], ss[sub][:], False)
                            act_ns += 1024 * 0.833 + 185
                    elif i0 == 4 * j:
                        # first diag k-step: idx0 full + idx1 off=128; exp the
                        # whole tile in one instr (the 128-col gap is below-
                        # diagonal garbage that PV never reads)
                        emit_exp(e[:], ss[sub][:], False)
                        act_ns += 1024 * 0.833 + 185
                    else:
                        for idx in (0, 1):
                            i = i0 + idx
                            off = max(0, 128 * i - 512 * j)
                            lo = 512 * idx
                            emit_exp(e[:, lo + off:lo + 512],
                                     ss[sub][:, lo + off:lo + 512], False)
                            act_ns += (512 - off) * 0.833 + 185
                    for idx in (0, 1):
                        i = i0 + idx
                        if i >= 4 * j:
                            off = 128 * i - 512 * j
                            win = e[:, 512 * idx + off:512 * idx + off + 128]
                            nc.gpsimd.tensor_mul(win, win, tri[:])
                    es[(sub, k)] = e
                yield act_ns, cover_ns, score_cyc

        def drain(g):
            for _ in g:
                pass

        def step(g):
            y = next(g, None)
            if y is not None:
                clk[0] += y[0]

        pv_tiles = {}

        def pv_alloc(p, j):
            pva = pvp.tile([128, 512], F32, tag="pvA", name=f"pva{p}{j}")
            pvb = pvp.tile([128, 512], F32, tag="pvB", name=f"pvb{p}{j}")
            pv_tiles[(p, j)] = (pva, pvb)
            return (pva, pva, pvb, pvb)

        def pv_block(p, j, es, bank, i):
            """PV matmuls for tk-block i of group (p,j)."""
            lo = 512 * (i % 2)
            kt = i // 2
            for tp in range(max(0, i - 4 * j), 4):
                for sub in (0, 1):
                    h = 2 * p + sub
                    kk = 2 * (tp % 2) + sub
                    nc.tensor.matmul(
                        out=bank[tp][:, 65 * kk:65 * kk + 65],
                        lhsT=es[(sub, kt)][:, lo + 128 * tp:lo + 128 * tp + 128],
                        rhs=vaug[i][:, 65 * h:65 * h + 65],
                        start=(i == 0 and tp % 2 == 0 and sub == 0),
                        stop=(i == 4 * j + tp and tp % 2 == 1 and sub == 1),
                    )

        def gen_pv(p, j, es):
            bank = pv_alloc(p, j)
            for i in range(4 * j + 4):
                pv_block(p, j, es, bank, i)

        def norm_tp(p, j, bank, ra, tp, name):
            o2 = attsm.tile([128, 128], F16, tag=f"o2n{tp}", bufs=3, name=name)
            for sub in (0, 1):
                kk = 2 * (tp % 2) + sub
                rcol = 4 * (tp // 2) + kk
                nc.vector.tensor_scalar_mul(
                    o2[:, 64 * sub:64 * sub + 64],
                    bank[tp][:, 65 * kk:65 * kk + 64],
                    ra[:, rcol:rcol + 1])
            t = 4 * j + tp
            nc.sync.dma_start_transpose(
                out=OF[p][:, 128 * t:128 * (t + 1)], in_=o2[:])

        def norm_recips(p, j):
            pva, pvb = pv_tiles.pop((p, j))
            bank = (pva, pva, pvb, pvb)
            ra = attsm.tile([128, 8], F32, tag="ra", bufs=3, name=f"ra{p}{j}")
            nc.vector.reciprocal(
                ra[:, 0:4].rearrange("p (k o) -> p k o", o=1),
                pva[:, 0:260].rearrange("p (k c) -> p k c", c=65)[:, :, 64:65])
            nc.vector.reciprocal(
                ra[:, 4:8].rearrange("p (k o) -> p k o", o=1),
                pvb[:, 0:260].rearrange("p (k c) -> p k c", c=65)[:, :, 64:65])
            return bank, ra

        def gen_norm(p, j):
            bank, ra = norm_recips(p, j)
            for tp in range(4):
                norm_tp(p, j, bank, ra, tp, f"o2n{p}{j}{tp}")

        # ACT-paced logical clock (ns): floors deferred-work start times so
        # the tile list-scheduler cannot hoist them into too-early PE bubbles
        # (its CoreSim model underestimates the DVE->DMA norm chain).
        clk = [5000.0]

        def emit_proj_eo(t, eo, el):
            ob = osb.tile([128, 512], F32, tag="ob", name=f"ob{t}_{eo}")
            ps = mm.tile([128, 512], F32, tag="mm", name=f"pps{t}_{eo}")
            with tc.tile_wait_until(clk[0] / 1e6):
                for p in range(3):
                    nc.tensor.matmul(
                        out=ps[:, 0:el],
                        lhsT=OF[p][:, 128 * t:128 * (t + 1)],
                        rhs=wp_b[:, 768 * p + eo:768 * p + eo + el],
                        start=(p == 0), stop=(p == 2),
                    )
                nc.vector.tensor_copy(ob[:, 0:el], ps[:, 0:el])
            nc.sync.dma_start(out=out[128 * t:128 * (t + 1), eo:eo + el],
                              in_=ob[:, 0:el])

        def emit_proj_t(t):
            for eo, el in ((0, 512), (512, 256)):
                emit_proj_eo(t, eo, el)

        # ---------------- input DMAs (progressive) ----------------
        xw = pp.tile([128, 4 * 6144], F8, tag="xw", name="xw")
        w8qk_t = pp.tile([128, 4608], F8, tag="w8qk", name="w8qk")
        wvhi_t = pp.tile([128, 2304], F8, tag="wvhi", name="wvhi")
        wvcr_t = pp.tile([128, 4608], F8, tag="wvcr", name="wvcr")
        wp_b = xtw.tile([128, 3 * C], F16, tag="wpb", name="wpb")

        # first-chunk-first DMA order: the prologue's qk(0,0)/qk(3,0) chains
        # start as soon as chunk-pair a of window 0 and its weights land
        for a in range(3):
            nc.sync.dma_start(out=w8qk_t[:, 1536 * a:1536 * (a + 1)],
                              in_=w8qk[:, 1536 * a:1536 * (a + 1)])
            nc.sync.dma_start(out=xw[:, 2048 * a:2048 * (a + 1)],
                              in_=x8w[:, 2048 * a:2048 * (a + 1)])
        nc.sync.dma_start(out=wvhi_t[:], in_=wvhi)
        nc.sync.dma_start(out=wvcr_t[:], in_=wvcr)
        nc.sync.dma_start(out=xw[:, 6144:12288], in_=x8w[:, 6144:12288])
        nc.sync.dma_start(
            out=wp_b[:].rearrange("p (c w) -> p c w", w=C),
            in_=wp[:, :].rearrange("(c p2) w -> p2 c w", c=3))
        nc.sync.dma_start(out=xw[:, 12288:18432], in_=x8w[:, 12288:18432])
        nc.sync.dma_start(out=xw[:, 18432:24576], in_=x8w[:, 18432:24576])

        def xw4(j):
            # [128, c(6), h(2), t(512)] view of window j
            return xw[:, 6144 * j:6144 * (j + 1)].rearrange(
                "p (c h t) -> p c h t", c=6, h=2)

        def emit_qk(m, j, on_act=False):
            """q/k tile (m<3: q pair m; m>=3: k pair m-3), window j: hi-only.
            on_act: route the plain conversions through the (prologue-idle)
            ACT engine instead of DVE."""
            ps = mm.tile([128, 512], F32, tag="mm", name=f"qkps{m}_{j}")
            x4 = xw4(j)
            for a in range(3):
                nc.tensor.matmul(
                    out=ps[:],
                    lhsT=w8qk_t[:, 1536 * a:1536 * (a + 1)]
                        .rearrange("p (s w) -> p s w", s=2)
                        [:, :, 128 * m:128 * m + 128],
                    rhs=x4[:, 2 * a:2 * a + 2, 0:1, :].squeeze(2),
                    start=(a == 0), stop=(a == 2), perf_mode=DR,
                )
            if m < 3:
                if on_act:
                    nc.scalar.mul(Qt[m][:, 512 * j:512 * (j + 1)], ps[:], WIN)
                else:
                    nc.vector.tensor_scalar_mul(
                        Qt[m][:, 512 * j:512 * (j + 1)], ps[:], WIN)
                nc.vector.scalar_tensor_tensor(
                    out=Qt[m][:, T + 512 * j:T + 512 * (j + 1)],
                    in0=ps[:], scalar=WIN,
                    in1=Qt[m][:, 512 * j:512 * (j + 1)],
                    op0=mybir.AluOpType.mult,
                    op1=mybir.AluOpType.subtract)
            else:
                if on_act:
                    nc.scalar.mul(Kt[m - 3][:, 512 * j:512 * (j + 1)], ps[:], WIN)
                else:
                    nc.vector.tensor_scalar_mul(
                        Kt[m - 3][:, 512 * j:512 * (j + 1)], ps[:], WIN)

        def emit_v(t):
            nc.gpsimd.memset(vaug[t][:], 1.0)
            ps = mm.tile([128, 512], F32, tag="mm", name=f"vps{t}")[:, 0:HPC * D]
            x4 = xw4(t // 4)
            tb = 128 * (t % 4)
            for a in range(3):
                nc.tensor.matmul(
                    out=ps,
                    lhsT=x4[:, 2 * a:2 * a + 2, 0:1, tb:tb + 128].squeeze(2),
                    rhs=wvhi_t[:, 768 * a:768 * (a + 1)]
                        .rearrange("p (s w) -> p s w", s=2),
                    start=(a == 0), stop=False, perf_mode=DR,
                )
            for c in range(NCC):
                nc.tensor.matmul(
                    out=ps,
                    lhsT=x4[:, c:c + 1, :, tb:tb + 128].squeeze(1),
                    rhs=wvcr_t[:, 768 * c:768 * (c + 1)]
                        .rearrange("p (s w) -> p s w", s=2),
                    start=False, stop=(c == NCC - 1), perf_mode=DR,
                )
            dst = vaug[t].rearrange("p (h c) -> p h c", c=D + 1)[:, :, 0:D]
            src = ps.rearrange("p (h c) -> p h c", c=D)
            nc.vector.tensor_scalar_mul(dst, src, WIN)

        # ------ qkv piece distribution: each hoisted group's qk tiles land
        # one iteration before the hoist needs them --------------------------
        PIECES = {
            0: [("qk", 1, 0), ("qk", 4, 0), ("v", 0), ("v", 1)],
            1: [("qk", 2, 0), ("qk", 5, 0), ("v", 2), ("v", 3)],
            2: [("qk", 0, 1), ("qk", 3, 1), ("v", 4)],
            3: [("qk", 1, 1), ("qk", 4, 1), ("v", 5), ("v", 6)],
            4: [("qk", 2, 1), ("qk", 5, 1), ("v", 7)],
            5: [("qk", 0, 2), ("qk", 3, 2), ("v", 8)],
            6: [("qk", 1, 2), ("qk", 4, 2), ("v", 9), ("v", 10)],
            7: [("qk", 2, 2), ("qk", 5, 2), ("v", 11)],
            8: [("qk", 0, 3), ("qk", 3, 3), ("v", 12)],
            9: [("qk", 1, 3), ("qk", 4, 3), ("v", 13), ("v", 14)],
            10: [("qk", 2, 3), ("qk", 5, 3), ("v", 15)],
            11: [],
        }

        def emit_pieces_qk(n):
            for pc in PIECES[n]:
                if pc[0] == "qk":
                    emit_qk(pc[1], pc[2])

        def emit_pieces_v(n):
            for pc in PIECES[n]:
                if pc[0] == "v":
                    emit_v(pc[1])

        # ---------------- unified pipeline --------------------------------
        # Deferred PE work queue: tasks (cost-in-PE-cycles, closure) slotted
        # into the ACT-covered window of each score k-step so the in-order PE
        # queue never blocks the score stream that feeds ACT.
        import collections as _co
        pe_q = _co.deque()

        def pump(budget):
            while pe_q and budget > 0 and pe_q[0][0] <= budget:
                cost, fn = pe_q.popleft()
                fn()
                budget -= cost

        def pump_all():
            while pe_q:
                pe_q.popleft()[1]()

        def pump_k(y):
            if y is None:
                return
            act_ns, cover_ns, score_cyc = y
            clk[0] += act_ns
            pump(max(0, int((act_ns + cover_ns) * 2.04) - score_cyc))

        def enq_piece(pc):
            if pc[0] == "qk":
                m, jj = pc[1], pc[2]
                pe_q.append((800, lambda m=m, jj=jj: emit_qk(m, jj)))
            else:
                t = pc[1]
                pe_q.append((1800, lambda t=t: emit_v(t)))

        def enq_pv(p, j, es):
            box = {}
            for i in range(4 * j + 4):
                ntp = 4 - max(0, i - 4 * j)

                def run(i=i):
                    if i == 0:
                        box["bank"] = pv_alloc(p, j)
                    pv_block(p, j, es, box["bank"], i)

                pe_q.append((ntp * 130, run))

        def enq_proj_t(t):
            for eo, el in ((0, 512), (512, 256)):
                pe_q.append((3 * el, lambda t=t, eo=eo, el=el:
                             emit_proj_eo(t, eo, el)))

        groups = [(p, j) for j in range(NJ) for p in range(3)]
        E = {}

        # prologue: q0/k0 for window 0 (conversions on the idle ACT engine)
        emit_qk(0, 0, on_act=True)
        emit_qk(3, 0, on_act=True)
        E[0] = {}
        cur = gen_ph1(0, 0, 0, E[0])
        step(cur)                      # k0 of group 0
        for n, (p, j) in enumerate(groups):
            if n < 11:
                for y in cur:          # rest of ph1(g_n), pumping PE work
                    pump_k(y)
                E[n + 1] = {}
                cur = gen_ph1(n + 1, *groups[n + 1], E[n + 1])
                # next group's q/k tiles must be emitted before its hoisted k0
                for pc in PIECES[n]:
                    if pc[0] == "qk":
                        emit_qk(pc[1], pc[2])
                pump_k(next(cur, None))  # k0 of g_{n+1}
                for pc in PIECES[n]:
                    if pc[0] == "v":
                        enq_piece(pc)
                if n >= 2:
                    gp, gj = groups[n - 2]
                    pe_q.append((0, lambda gp=gp, gj=gj: gen_norm(gp, gj)))
                if n >= 1:
                    enq_pv(*groups[n - 1], E[n - 1])
                # projections spread over the later iterations, each at least
                # one iteration after its level's last norm has popped (the
                # norm->transpose->OF chain has ~2.4us DMA latency)
                if n in (6, 7, 9):
                    for t in {6: (0, 1), 7: (2, 3), 9: (4, 5, 6, 7)}[n]:
                        enq_proj_t(t)
            else:
                # last group (2,3): fuse pv(2,3) into its own score/exp loop,
                # and normalize/project each PSUM bank as soon as it stops
                step(cur)              # k1
                pump_all()             # drain any deferred PE backlog
                step(cur)              # k2
                gen_norm(*groups[n - 2])
                gen_pv(*groups[n - 1], E[n - 1])        # pv(1,3)
                gen_norm(*groups[n - 1])                # norm(1,3)
                bank = pv_alloc(p, j)

                def tail_norm_bank(btile, tps):
                    for tp in tps:
                        ra = attsm.tile([128, 8], F32, tag="ra", bufs=3,
                                        name=f"raT{tp}")
                        k0_ = 2 * (tp % 2)
                        nc.vector.reciprocal(
                            ra[:, k0_:k0_ + 2].rearrange("p (k o) -> p k o", o=1),
                            btile[:, 65 * k0_:65 * k0_ + 130].rearrange(
                                "p (k c) -> p k c", c=65)[:, :, 64:65])
                        o2 = attsm.tile([128, 128], F16, tag=f"o2n{tp}", bufs=3,
                                        name=f"o2nT{tp}")
                        for sub in (0, 1):
                            kk = 2 * (tp % 2) + sub
                            if sub == 0:
                                # ACT is idle after the last exp: split the two
                                # scale-converts across ACT and DVE
                                nc.scalar.mul(
                                    o2[:, 0:64],
                                    btile[:, 65 * kk:65 * kk + 64],
                                    ra[:, kk:kk + 1])
                            else:
                                nc.vector.tensor_scalar_mul(
                                    o2[:, 64 * sub:64 * sub + 64],
                                    btile[:, 65 * kk:65 * kk + 64],
                                    ra[:, kk:kk + 1])
                        t = 4 * j + tp
                        # PE transpose (latency ~0.1us vs ~2.4us DMA xbar);
                        # copies ride the now-idle ACT engine
                        tps_ = mm.tile([128, 512], F32, tag="mm",
                                       name=f"tps{tp}")[:, 0:64].bitcast(F16)
                        nc.tensor.matmul(out=tps_, lhsT=o2[:], rhs=iden[:],
                                         is_transpose=True)
                        nc.scalar.copy(OF[p][:, 128 * t:128 * (t + 1)], tps_)
                        ob = osb.tile([128, C], F32, tag="ob", name=f"obT{t}")
                        for eo, el in ((0, 512), (512, 256)):
                            ps = mm.tile([128, 512], F32, tag="mm",
                                         name=f"ppsT{t}_{eo}")
                            for pr in range(3):
                                nc.tensor.matmul(
                                    out=ps[:, 0:el],
                                    lhsT=OF[pr][:, 128 * t:128 * (t + 1)],
                                    rhs=wp_b[:, 768 * pr + eo:768 * pr + eo + el],
                                    start=(pr == 0), stop=(pr == 2),
                                )
                            if eo == 0:
                                nc.scalar.copy(ob[:, 0:512], ps[:, 0:512])
                            else:
                                nc.vector.tensor_copy(ob[:, eo:eo + el],
                                                      ps[:, 0:el])
                            nc.sync.dma_start(
                                out=out[128 * t:128 * (t + 1), eo:eo + el],
                                in_=ob[:, eo:eo + el])

                pvb_ = bank[2]

                def pv_block_a(i):
                    # [tq,65]-orientation blocks for t'=0,1 only (bank A)
                    lo = 512 * (i % 2)
                    kt = i // 2
                    for tp in range(max(0, i - 12), 2):
                        for sub in (0, 1):
                            h = 2 * p + sub
                            kk = 2 * tp + sub
                            nc.tensor.matmul(
                                out=bank[0][:, 65 * kk:65 * kk + 65],
                                lhsT=E[n][(sub, kt)][:, lo + 128 * tp:lo + 128 * tp + 128],
                                rhs=vaug[i][:, 65 * h:65 * h + 65],
                                start=(i == 0 and tp == 0 and sub == 0),
                                stop=(i == 12 + tp and tp == 1 and sub == 1),
                            )

                def pv_slab(i):
                    # t14/t15 in [65,tq] orientation: O^T slab [65, 2x256]
                    lo = 512 * (i % 2)
                    kt = i // 2
                    off = max(0, 128 * i - 1792)
                    for sub in (0, 1):
                        h = 2 * p + sub
                        nc.tensor.matmul(
                            out=pvb_[0:65, 256 * sub + off:256 * sub + 256],
                            lhsT=vaug[i][:, 65 * h:65 * h + 65],
                            rhs=E[n][(sub, kt)][:, lo + 256 + off:lo + 512],
                            start=(i == 0 and sub == 0),
                            stop=(i == 15 and sub == 1),
                        )

                nb = 0
                for k in range(3, 8):
                    step(cur)                           # k-step k
                    if 3 <= k <= 6:
                        emit_proj_t(k + 5)              # t8..t11
                    while nb <= 2 * k - 1:
                        pv_block_a(nb)
                        pv_slab(nb)
                        nb += 1
                        if nb == 14:                    # bank A (t' 0,1) stopped
                            tail_norm_bank(bank[0], (0, 1))
                for i in range(nb, 16):
                    pv_block_a(i)
                    pv_slab(i)
                pv_tiles.pop((p, j))
                # slab normalize: recip row + partition_broadcast + scale into
                # OF, then project t14/t15
                rrow = attsm.tile([1, 512], F32, tag="rrow", name="rrowT")
                rb = attsm.tile([64, 512], F32, tag="rbT", name="rbT")
                nc.vector.reciprocal(rrow[0:1, :], pvb_[64:65, :])
                nc.gpsimd.partition_broadcast(rb[0:64, :], rrow[0:1, :])
                for sub in (0, 1):
                    nc.vector.tensor_mul(
                        OF[p][64 * sub:64 * sub + 64, 1792:2048],
                        pvb_[0:64, 256 * sub:256 * sub + 256],
                        rb[0:64, 256 * sub:256 * sub + 256])
                for t in (14, 15):
                    ob = osb.tile([128, C], F32, tag="ob", name=f"obT{t}")
                    for eo, el in ((0, 512), (512, 256)):
                        ps = mm.tile([128, 512], F32, tag="mm",
                                     name=f"ppsT{t}_{eo}")
                        for pr in range(3):
                            nc.tensor.matmul(
                                out=ps[:, 0:el],
                                lhsT=OF[pr][:, 128 * t:128 * (t + 1)],
                                rhs=wp_b[:, 768 * pr + eo:768 * pr + eo + el],
                                start=(pr == 0), stop=(pr == 2),
                            )
                        if eo == 0:
                            nc.scalar.copy(ob[:, 0:512], ps[:, 0:512])
                        else:
                            nc.vector.tensor_copy(ob[:, eo:eo + el],
                                                  ps[:, 0:el])
                        nc.sync.dma_start(
                            out=out[128 * t:128 * (t + 1), eo:eo + el],
                            in_=ob[:, eo:eo + el])


_NC_CACHE = None


def build_nc():
    global _NC_CACHE
    if _NC_CACHE is not None:
        return _NC_CACHE
    nc = bacc.Bacc(trn_type="TRN2")
    x8w = nc.dram_tensor("x8w", [128, 24576], F8, kind="ExternalInput").ap()
    w8qk = nc.dram_tensor("w8qk", [128, 4608], F8, kind="ExternalInput").ap()
    wvhi = nc.dram_tensor("wvhi", [128, 2304], F8, kind="ExternalInput").ap()
    wvcr = nc.dram_tensor("wvcr", [128, 4608], F8, kind="ExternalInput").ap()
    wp = nc.dram_tensor("wp", [HPC * D, C], F16, kind="ExternalInput").ap()
    out = nc.dram_tensor("out", [T, C], F32, kind="ExternalOutput").ap()
    with tile.TileContext(nc) as tc:
        _emit(nc, tc, x8w, w8qk, wvhi, wvcr, wp, out)
    nc.compile()
    _NC_CACHE = nc
    return nc


def _f8(a):
    return np.asarray(a, dtype=NP8)


def make_in_maps(x, W_attn, W_proj):
    x = np.asarray(x, dtype=np.float32)
    W_attn = np.asarray(W_attn, dtype=np.float32)
    W_proj = np.asarray(W_proj, dtype=np.float32)
    W64 = 64.0 * W_attn
    W8 = _f8(W64)
    W8r = _f8(W64 - W8.astype(np.float32))
    in_maps = []
    for core in range(8):
        b = core // 2
        h0 = HPC * (core % 2)
        xT = np.ascontiguousarray(x[b].T)            # [C, T]
        x8 = _f8(xT)
        x8r = _f8(xT - x8.astype(np.float32))
        # x8w[p, 6144j + 1024c + 512h + t] = (x8,x8r)[h][128c+p, 512j+t]
        st = np.stack([x8, x8r], axis=0).reshape(2, 6, 128, 4, 512)
        x8w = np.ascontiguousarray(
            st.transpose(2, 3, 1, 0, 4).reshape(128, 24576))
        # q,k hi weights (no q pre-scale; 1/sqrt(d) folds into exp scale)
        qcols = W8[:, 64 * h0:64 * h0 + 384]
        kcols = W8[:, 768 + 64 * h0:768 + 64 * h0 + 384]
        wqk = np.concatenate([qcols.astype(np.float32),
                              kcols.astype(np.float32)], axis=1)
        w8qk = np.ascontiguousarray(
            _f8(wqk).reshape(3, 2, 128, 768).transpose(2, 0, 1, 3)
            .reshape(128, 4608))
        vhi = W8[:, 1536 + 64 * h0:1536 + 64 * h0 + 384]
        vlo = W8r[:, 1536 + 64 * h0:1536 + 64 * h0 + 384]
        wvhi = np.ascontiguousarray(
            vhi.reshape(3, 2, 128, 384).transpose(2, 0, 1, 3)
            .reshape(128, 2304))
        # cross tile per chunk c: slot0 = W8r_c (pairs x8), slot1 = W8_c (pairs x8r)
        wvcr = np.ascontiguousarray(
            np.stack([vlo.reshape(6, 128, 384), vhi.reshape(6, 128, 384)],
                     axis=1).transpose(2, 0, 1, 3).reshape(128, 4608))
        wp_ = np.ascontiguousarray(
            W_proj[64 * h0:64 * h0 + 384, :]).astype(np.float16)
        in_maps.append({"x8w": x8w, "w8qk": w8qk, "wvhi": wvhi,
                        "wvcr": wvcr, "wp": wp_})
    return in_maps


def kernel(x, W_attn, W_proj, _trace=False, _trace_kwargs=None):
    nc = build_nc()
    in_maps = make_in_maps(x, W_attn, W_proj)
    res = run_bass_kernel_spmd(nc, in_maps, list(range(8)), trace=_trace,
                               **(_trace_kwargs or {}))
    outs = [res.results[c]["out"] for c in range(8)]
    y = np.stack([outs[2 * b] + outs[2 * b + 1] for b in range(4)]).astype(np.float32)
    if _trace:
        return y, res
    return y
